# revision 39
# baseline (speedup 1.0000x reference)
"""Trainium2 Bass kernel for causal multi-head attention + output projection.

Problem (hardcoded): x[4, 2048, 1024] fp32, 16 heads, head_dim 64, causal,
torch-Linear convention (y = x @ W.T), output projection with bias.

Sharding over 8 NeuronCores: batch (4) x head-group (2 groups of 8 heads).
Each core computes q/k/v for its 8 heads of its batch, causal attention in
the S^T layout (keys on partitions, queries on free dim; softmax denominators
produced by an appended ones-column in V), then a PARTIAL output projection
over its own 8 heads (contraction 512) for all 2048 queries. The host sums
the two partial projections of each batch (the tensor-parallel all-reduce,
done host-side because on-device collectives on this fabric run ~30 GB/s);
ATTN_COMBINE=rs switches to an on-device chunked pairwise ReduceScatter.

Single fused pipeline: attention steps t=(head, qh, j) emit S(t) on the PE
and exp(t) on ACT, with AV(t-1) lagged one step. The QKV projection is NOT a
separate phase: K/Q/V tile computations are interleaved as "filler" PE work
inside the attention step stream (the attention loop is ACT-bound, so the PE
slack absorbs them), sharing the PSUM s-ring. kT is stored zero-padded per
head on the full 128 partitions so every matmul in the kernel runs in the
same (128,128) PE tile mode — mode switches would expose every LDWEIGHTS.

Everything is bf16 into fp32 PSUM; partial projections are exchanged fp32.
"""
import os
import sys
import types

import numpy as np

import concourse.bass as bass
import concourse.mybir as mybir
import concourse.tile as tile
from concourse import bacc, bass_utils

DT = getattr(mybir.dt, os.environ.get("ATTN_DT_MAIN", "bfloat16"))
F32 = mybir.dt.float32
AF = mybir.ActivationFunctionType
OP = mybir.AluOpType

B, T, D = 4, 2048, 1024
H, HD = 16, 64
HG = 8          # heads per core
QH = T // 2     # query half
N_CORES = 8
SCALE = 1.0 / 8.0

ADT_NAME = os.environ.get("ATTN_DTYPE", "bfloat16")
ADT = getattr(mybir.dt, ADT_NAME)
COMBINE = os.environ.get("ATTN_COMBINE", "hostsum")

RG_PAIRS = [[0, 1], [2, 3], [4, 5], [6, 7]]


# ---------------------------------------------------------------------------
# environment glue
# ---------------------------------------------------------------------------

def _install_ntff_hook():
    if 'antenv.axon_hooks' in sys.modules:
        return
    try:
        from trn_agent_boot.trn_boot import _ntff_profile_via_ctypes
        hook = _ntff_profile_via_ctypes('/opt/axon/libaxon_pjrt.so')
    except Exception:
        hook = None
    mod = types.ModuleType('antenv.axon_hooks')
    mod.get_axon_ntff_profile_hook = lambda: hook
    mod.set_axon_ntff_profile_hook = lambda h: None
    sys.modules['antenv.axon_hooks'] = mod


def _run_spmd(nc, in_maps, trace=False):
    from concourse.bass_interp import get_hw_module
    bass_utils.upload_artifacts = lambda tmpdir: tmpdir
    if trace:
        _install_ntff_hook()
    old_m = nc.m
    nc.m = get_hw_module(nc.m)
    try:
        return bass_utils.run_bass_kernel_spmd(
            nc, in_maps, core_ids=list(range(N_CORES)),
            trace=trace, trace_cores=[0] if trace else None,
        )
    finally:
        nc.m = old_m


# ---------------------------------------------------------------------------
# kernel program
# ---------------------------------------------------------------------------

def build_nc(combine):
    nc = bacc.Bacc("TRN2", target_bir_lowering=False, debug=False,
                   enable_asserts=False, num_devices=N_CORES)
    xT = nc.dram_tensor("xT", [D, T], DT, kind="ExternalInput").ap()
    wqT = nc.dram_tensor("wqT", [D, 512], DT, kind="ExternalInput").ap()
    wkT = nc.dram_tensor("wkT", [D, 512], DT, kind="ExternalInput").ap()
    wvT = nc.dram_tensor("wvT", [D, 512], DT, kind="ExternalInput").ap()
    wpT = nc.dram_tensor("wpT", [512, D], DT, kind="ExternalInput").ap()
    mask = nc.dram_tensor("mask", [128, 128], ADT, kind="ExternalInput").ap()
    zro = nc.dram_tensor("zro", [1, T], ADT, kind="ExternalInput").ap()
    bias = nc.dram_tensor("bias", [1, D], F32, kind="ExternalInput").ap()
    snum = nc.dram_tensor("snum", [32, QH], F32).ap()
    srec = nc.dram_tensor("srec", [32, QH], ADT).ap()
    if combine == "rs":
        rsin = [nc.dram_tensor(f"rsin{i}", [2, 128, D], F32).ap()
                for i in range(8)]
        yint = nc.dram_tensor("yint", [8, 128, D], F32).ap()
        yo = nc.dram_tensor("yo", [8, 128, D], F32, kind="ExternalOutput").ap()
    else:
        yo = nc.dram_tensor("yo", [T, D], F32, kind="ExternalOutput").ap()

    from contextlib import ExitStack
    with tile.TileContext(nc) as tc, ExitStack() as ctx:
        per = ctx.enter_context(tc.tile_pool(name="per", bufs=1))

        qT_sb = per.tile([128, 4, T], ADT, tag="qT")
        kT_sb = per.tile([128, 8, T], ADT, tag="kT")
        v_sb = per.tile([128, 16, HG, 65], ADT, tag="v")
        mask_sb = per.tile([128, 128], ADT, tag="mask")
        o_all = per.tile([128, 4, T], ADT, tag="oacc")
        wp_sb = per.tile([128, 4, D], DT, tag="wp")
        bias_bc = per.tile([128, D], F32, tag="bbc")

        nc.sync.dma_start(mask_sb[:], mask[:])

        # zero the dead half of a head's kT slot via a broadcast DMA
        # (gpsimd memset signals completion unreliably -> races with S reads)
        def emit_zro(h):
            dead = slice(64, 128) if h % 2 == 0 else slice(0, 64)
            nc.sync.dma_start(kT_sb[dead, h, :],
                              zro[0][None, :].broadcast_to([64, T]))

        xp = ctx.enter_context(tc.tile_pool(name="xph", bufs=16))
        wpo = ctx.enter_context(tc.tile_pool(name="wph", bufs=24))
        xT_r = xT.rearrange("(ko ki) t -> ki ko t", ki=128)

        def load_w(wT, nm):
            parts = []
            wT_r = wT.rearrange("(ko ki) n -> ki ko n", ki=128)
            for kk in range(8):
                t = wpo.tile([128, 512], DT, tag="w", name=f"{nm}{kk}")
                nc.sync.dma_start(t[:], wT_r[:, kk])
                parts.append(t)
            return parts

        def load_x(half):
            out = []
            for kk in range(8):
                t = xp.tile([128, QH], DT, tag="xh", name=f"x{half}_{kk}")
                nc.sync.dma_start(t[:], xT_r[:, kk, half * QH:(half + 1) * QH])
                out.append(t)
            return out

        # m0-sliced weight loads: tiny DMAs so K/Q for heads 0,1 half0 can
        # start ~1us in, unblocking the first exps while the bulk loads run.
        wm0 = ctx.enter_context(tc.tile_pool(name="wm0", bufs=16))
        wkT_r = wkT.rearrange("(ko ki) n -> ki ko n", ki=128)
        wqT_r = wqT.rearrange("(ko ki) n -> ki ko n", ki=128)
        wk_m0, wq_m0 = [], []
        for kk in range(8):
            t = wm0.tile([128, 128], DT, tag="wm", name=f"wkm{kk}")
            nc.sync.dma_start(t[:], wkT_r[:, kk, 0:128])
            wk_m0.append(t)
        for kk in range(8):
            t = wm0.tile([128, 128], DT, tag="wm", name=f"wqm{kk}")
            nc.sync.dma_start(t[:], wqT_r[:, kk, 0:128])
            wq_m0.append(t)

        # bulk input DMAs in consumption order
        xh = [None, None]
        xh[0] = load_x(0)
        wk_sb = load_w(wkT, "wk")
        wq_sb = load_w(wqT, "wq")
        emit_zro(0)
        emit_zro(1)
        xh[1] = load_x(1)
        wv_sb = load_w(wvT, "wv")
        nc.sync.dma_start(wp_sb[:],
                          wpT.rearrange("(ko ki) n -> ki ko n", ki=128))
        nc.sync.dma_start(bias_bc[:], bias[0][None, :].broadcast_to([128, D]))

        yop = ctx.enter_context(tc.tile_pool(name="yop", bufs=3))
        with ExitStack() as attn:
            sps = attn.enter_context(tc.tile_pool(name="sps", bufs=2, space="PSUM"))
            ops = attn.enter_context(tc.tile_pool(name="ops", bufs=2, space="PSUM"))
            es = attn.enter_context(tc.tile_pool(name="es", bufs=10))
            ev = attn.enter_context(tc.tile_pool(name="ev", bufs=2))
            nrm = attn.enter_context(tc.tile_pool(name="nrm", bufs=2))

            # ---- filler units: K/Q/V tile computations fed into PE slack --
            # units are ~1.7us (one 512-query sweep) so they never starve ACT
            def emit_K(m, half, nch, wtiles=None):
                # kT for heads (2m, 2m+1), zero-padded layout
                pt = sps.tile([128, 512], F32, tag="s",
                              name=f"ptk{m}_{half}_{nch}")
                sl = slice(nch * 512, (nch + 1) * 512)
                for kk in range(8):
                    lhsT = (wtiles[kk][:] if wtiles is not None
                            else wk_sb[kk][:, m * 128:(m + 1) * 128])
                    nc.tensor.matmul(
                        pt[:], lhsT=lhsT, rhs=xh[half][kk][:, sl],
                        start=(kk == 0), stop=(kk == 7))
                tsl = slice(half * QH + nch * 512, half * QH + nch * 512 + 512)
                nc.vector.tensor_copy(kT_sb[0:64, 2 * m, tsl], pt[0:64, :])
                nc.vector.tensor_copy(kT_sb[64:128, 2 * m + 1, tsl],
                                      pt[64:128, :])

            def emit_Q(m, half, nch, wtiles=None):
                pt = sps.tile([128, 512], F32, tag="s",
                              name=f"ptq{m}_{half}_{nch}")
                sl = slice(nch * 512, (nch + 1) * 512)
                for kk in range(8):
                    lhsT = (wtiles[kk][:] if wtiles is not None
                            else wq_sb[kk][:, m * 128:(m + 1) * 128])
                    nc.tensor.matmul(
                        pt[:], lhsT=lhsT, rhs=xh[half][kk][:, sl],
                        start=(kk == 0), stop=(kk == 7))
                tsl = slice(half * QH + nch * 512, half * QH + nch * 512 + 512)
                nc.vector.tensor_copy(qT_sb[:, m, tsl], pt[:])

            def emit_V(m):
                # V' for key block m, all 8 heads (+ ones column for the
                # softmax denominators)
                pt = sps.tile([128, QH], F32, tag="s", name=f"ptv{m}")
                for kk in range(8):
                    nc.tensor.matmul(
                        pt[:, 0:512],
                        lhsT=xh[m // 8][kk][:, (m % 8) * 128:(m % 8 + 1) * 128],
                        rhs=wv_sb[kk][:],
                        start=(kk == 0), stop=(kk == 7))
                nc.vector.tensor_copy(
                    v_sb[:, m, :, 0:64],
                    pt[:, 0:512].rearrange("p (h d) -> p h d", h=HG))
                nc.vector.memset(v_sb[:, m, :, 64], 1.0)

            # ---- attention steps -----------------------------------------
            # Head h's qh0 triangle is interleaved 2:1 into head (h-1)'s qh1
            # block: smooths the exp sizes ACT sees, removes head-transition
            # bubbles, and finishes every qh0 norm before the last qh1 block
            # (so half the projection can fill the drain).
            steps = [(0, 0, j) for j in range(8)]
            for h in range(HG - 1):
                for i in range(8):
                    steps.append((h, 1, 2 * i))
                    steps.append((h, 1, 2 * i + 1))
                    steps.append((h + 1, 0, i))
            steps += [(HG - 1, 1, j) for j in range(16)]
            n_steps = len(steps)

            e_tiles = [None] * n_steps
            o_tiles = {}

            def emit_S_exp(t):
                h, qh, j = steps[t]
                qstart = max(QH * qh, 128 * j)
                n = QH * (qh + 1) - qstart
                sub = h // 2
                s_ps = sps.tile([128, QH], F32, tag="s", name=f"s{t}")
                for c in range(0, n, 512):
                    cn = min(512, n - c)
                    nc.tensor.matmul(
                        s_ps[:, c:c + cn],
                        lhsT=kT_sb[:, h, j * 128:(j + 1) * 128],
                        rhs=qT_sb[:, sub, qstart + c:qstart + c + cn],
                        start=True, stop=True)
                e_sb = es.tile([128, QH], ADT, tag="e", name=f"e{t}")
                nc.scalar.activation(e_sb[:, 0:n], s_ps[:, 0:n], AF.Exp,
                                     scale=SCALE)
                if j >= 8 * qh:
                    nc.vector.tensor_tensor(
                        e_sb[:, 0:128], e_sb[:, 0:128], mask_sb[:], OP.mult)
                e_tiles[t] = (e_sb, n, qstart - QH * qh)

            def emit_AV(t):
                h, qh, j = steps[t]
                e_sb, n, coff = e_tiles[t]
                if j == 0:
                    o_tiles[(h, qh)] = ops.tile([65, QH], F32, tag="o",
                                                name=f"o{h}_{qh}")
                o_ps = o_tiles[(h, qh)]
                jmax = 8 * qh + 8
                c0 = coff
                while c0 < QH:
                    hi = min(QH, (c0 // 512 + 1) * 512)
                    nc.tensor.matmul(
                        o_ps[:, c0:hi],
                        lhsT=v_sb[:, j, h, :],
                        rhs=e_sb[:, c0 - coff:hi - coff],
                        start=(j == 0), stop=(j == jmax - 1),
                        skip_group_check=True)
                    c0 = hi
                e_tiles[t] = None

            def emit_evict(h, qh):
                o_ps = o_tiles.pop((h, qh))
                pbase = 64 * (h % 2)
                sub = h // 2
                i = 2 * h + qh
                dtile = ev.tile([1, QH], F32, tag="dn", name=f"dn{h}_{qh}")
                nc.vector.tensor_copy(dtile[:], o_ps[64:65, :])
                nc.sync.dma_start(snum[i:i + 1, :], dtile[:])
                if pbase == 0:
                    nc.vector.tensor_copy(
                        o_all[0:64, sub, QH * qh:QH * (qh + 1)], o_ps[0:64, :])
                else:
                    tmp = ev.tile([64, QH], ADT, tag="ev", name=f"ev{h}_{qh}")
                    nc.vector.tensor_copy(tmp[:], o_ps[0:64, :])
                    nc.sync.dma_start(
                        o_all[64:128, sub, QH * qh:QH * (qh + 1)], tmp[:])
                st64 = nrm.tile([64, QH // 64], F32, tag="sp")
                nc.sync.dma_start(st64[:], snum[i].rearrange("(p f) -> p f", p=64))
                nc.vector.reciprocal(st64[:], st64[:])
                st64b = nrm.tile([64, QH // 64], ADT, tag="spb")
                nc.vector.tensor_copy(st64b[:], st64[:])
                nc.sync.dma_start(srec[i].rearrange("(p f) -> p f", p=64), st64b[:])
                bc = nrm.tile([128, QH], ADT, tag="bc")
                nc.sync.dma_start(bc[pbase:pbase + 64, :],
                                  srec[i][None, :].broadcast_to([64, QH]))
                sl_ap = o_all[pbase:pbase + 64, sub, QH * qh:QH * (qh + 1)]
                nc.vector.tensor_tensor(sl_ap, sl_ap, bc[pbase:pbase + 64, :],
                                        OP.mult)

            # ---- filler schedule: emission position -> list of closures --
            fillers = {}

            def add_filler(pos, fn, *args):
                fillers.setdefault(pos, []).append((fn, args))

            # ---- projection half-tiles (shared psum ring) ----------------
            def emit_proj(m, nch, pool, ptag="s"):
                yp = pool.tile([128, 512], F32, tag=ptag, name=f"yp{m}_{nch}")
                sl = slice(nch * 512, (nch + 1) * 512)
                for kk in range(4):
                    nc.tensor.matmul(
                        yp[:], lhsT=o_all[:, kk, m * 128:(m + 1) * 128],
                        rhs=wp_sb[:, kk, sl],
                        start=(kk == 0), stop=(kk == 3))
                y_sb = yop.tile([128, 512], F32, tag="y")
                nc.vector.tensor_tensor(y_sb[:], yp[:], bias_bc[:, sl], OP.add)
                if combine == "rs":
                    nc.sync.dma_start(rsin[m % 8][m // 8][:, sl], y_sb[:])
                else:
                    nc.sync.dma_start(yo[m * 128:(m + 1) * 128, sl], y_sb[:])

            # AV is lagged LAG steps behind S/exp so it never waits on ACT
            # and its ldweights hides under the S streams; this also relaxes
            # every V-filler deadline so QKV work spreads across the span.
            LAG = 8
            for j, pos in zip(range(8), (1, 2, 4, 5, 7, 9, 11, 13)):
                add_filler(pos, emit_V, j)              # AV(h0,qh0,j) at j+LAG
            for idx, j in enumerate(range(8, 16)):
                add_filler(15 + 3 * idx, emit_V, j)     # AV(h0,qh1,j) at j+8+LAG
            add_filler(3, emit_Q, 0, 1, 0)              # h0 qh1 from step 8
            add_filler(5, emit_Q, 0, 1, 1)
            add_filler(8, emit_K, 0, 1, 0)              # h0 qh1 j>=8 by step 20
            add_filler(12, emit_K, 0, 1, 1)
            for p in (1, 2, 3):
                b = 48 * p
                add_filler(b - 34, emit_K, p, 0, 0)     # by step 48p-14
                add_filler(b - 31, emit_K, p, 0, 1)
                add_filler(b - 26, emit_Q, p, 0, 0)     # by step 48p-14
                add_filler(b - 22, emit_Q, p, 0, 1)
                add_filler(b - 2, emit_Q, p, 1, 0)      # by step 48p+8
                add_filler(b + 2, emit_Q, p, 1, 1)
                add_filler(b + 6, emit_K, p, 1, 0)      # by step 48p+20
                add_filler(b + 10, emit_K, p, 1, 1)
            for h in range(2, HG):
                add_filler(24 * h - 20, emit_zro, h)    # by step 24h-14
            for idx in range(16):
                add_filler(184 + idx // 2, emit_proj,
                           idx // 2, idx % 2, sps)      # qh0 norms @~183

            # ---- fused emission ------------------------------------------
            emit_K(0, 0, 0, wk_m0)
            emit_K(0, 0, 1, wk_m0)
            emit_Q(0, 0, 0, wq_m0)
            emit_Q(0, 0, 1, wq_m0)
            for t in range(n_steps):
                emit_S_exp(t)
                for fn, args in fillers.get(t, []):
                    fn(*args)
                if t >= LAG:
                    emit_AV(t - LAG)
                    h0, qh0, j0 = steps[t - LAG]
                    if j0 == 8 * qh0 + 7:
                        emit_evict(h0, qh0)
            for t in range(n_steps - LAG, n_steps):
                emit_AV(t)
                h0, qh0, j0 = steps[t]
                if j0 == 8 * qh0 + 7:
                    emit_evict(h0, qh0)

        # tail projection in two passes: kk 0-2 (heads 0-5, whose norms land
        # early) prefill PSUM; the last-norm-gated kk=3 goes per tile so m>8
        # tiles aren't queued behind it.
        pps = ctx.enter_context(tc.tile_pool(name="pps", bufs=3, space="PSUM"))

        def tproj_pass1(m, nch):
            yp = pps.tile([128, 512], F32, tag="yp", name=f"tp{m}_{nch}")
            sl = slice(nch * 512, (nch + 1) * 512)
            for kk in range(3):
                nc.tensor.matmul(
                    yp[:], lhsT=o_all[:, kk, m * 128:(m + 1) * 128],
                    rhs=wp_sb[:, kk, sl],
                    start=(kk == 0), stop=False, skip_group_check=True)
            return yp

        def tproj_pass2(m, nch, yp):
            sl = slice(nch * 512, (nch + 1) * 512)
            nc.tensor.matmul(
                yp[:], lhsT=o_all[:, 3, m * 128:(m + 1) * 128],
                rhs=wp_sb[:, 3, sl],
                start=False, stop=True, skip_group_check=True)
            y_sb = yop.tile([128, 512], F32, tag="y")
            nc.vector.tensor_tensor(y_sb[:], yp[:], bias_bc[:, sl], OP.add)
            if combine == "rs":
                nc.sync.dma_start(rsin[m % 8][m // 8][:, sl], y_sb[:])
            else:
                nc.sync.dma_start(yo[m * 128:(m + 1) * 128, sl], y_sb[:])

        tail = [(m, nch) for m in range(8, 16) for nch in range(2)]
        live = []
        for m, nch in tail[:3]:
            live.append((m, nch, tproj_pass1(m, nch)))
        nxt = 3
        while live:
            m, nch, yp = live.pop(0)
            tproj_pass2(m, nch, yp)
            if nxt < len(tail):
                m2, n2 = tail[nxt]
                live.append((m2, n2, tproj_pass1(m2, n2)))
                nxt += 1
        if combine == "rs":
            for i in range(8):
                nc.gpsimd.collective_compute(
                    "ReduceScatter", OP.add,
                    replica_groups=RG_PAIRS,
                    ins=[rsin[i][:]], outs=[yint[i]],
                )
                nc.sync.dma_start(yo[i], yint[i])

    nc.compile()
    return nc


# ---------------------------------------------------------------------------
# host-side sharding + entry point
# ---------------------------------------------------------------------------

_NC_CACHE = {}


def _get_nc(combine):
    if combine not in _NC_CACHE:
        _NC_CACHE[combine] = build_nc(combine)
    return _NC_CACHE[combine]


def _make_in_maps(x, Wq, Wk, Wv, Wp, bp):
    x = np.asarray(x, dtype=np.float32)
    Wq = np.asarray(Wq, dtype=np.float32)
    Wk = np.asarray(Wk, dtype=np.float32)
    Wv = np.asarray(Wv, dtype=np.float32)
    Wp = np.asarray(Wp, dtype=np.float32)
    bp = np.asarray(bp, dtype=np.float32)

    adt_np = mybir.dt.np(ADT)
    dt_np = mybir.dt.np(DT)
    mask = np.zeros((128, 128), dtype=np.float32)
    k_idx = np.arange(128)[:, None]
    q_idx = np.arange(128)[None, :]
    mask[q_idx >= k_idx] = 1.0
    mask = mask.astype(adt_np)

    xTs = [np.ascontiguousarray(x[b].T) for b in range(B)]
    in_maps = []
    for c in range(N_CORES):
        b, g = c // 2, c % 2
        rows = slice(512 * g, 512 * (g + 1))
        m = {
            "xT": xTs[b].astype(dt_np),
            "wqT": np.ascontiguousarray(Wq[rows, :].T).astype(dt_np),
            "wkT": np.ascontiguousarray(Wk[rows, :].T).astype(dt_np),
            "wvT": np.ascontiguousarray(Wv[rows, :].T).astype(dt_np),
            "wpT": np.ascontiguousarray(Wp[:, rows].T).astype(dt_np),
            "mask": mask,
            "zro": np.zeros((1, T), dtype=adt_np),
            "bias": (bp if g == 0 else np.zeros_like(bp)).reshape(1, D),
        }
        in_maps.append(m)
    return in_maps


def kernel(x, Wq, Wk, Wv, Wp, bp, _trace=False):
    combine = COMBINE
    nc = _get_nc(combine)
    in_maps = _make_in_maps(x, Wq, Wk, Wv, Wp, bp)
    res = _run_spmd(nc, in_maps, trace=_trace)
    out = np.empty((B, T, D), dtype=np.float32)
    for b in range(B):
        ya = res.results[2 * b]["yo"]
        yb = res.results[2 * b + 1]["yo"]
        if combine == "rs":
            out[b, 0:QH] = ya.reshape(QH, D)
            out[b, QH:T] = yb.reshape(QH, D)
        else:
            out[b] = ya + yb
    if _trace:
        kernel.last_results = res
    return out


# revision 44
# speedup vs baseline: 1.0649x; 1.0649x over previous
"""Trainium2 Bass kernel for causal multi-head attention + output projection.

Problem (hardcoded): x[4, 2048, 1024] fp32, 16 heads, head_dim 64, causal,
torch-Linear convention (y = x @ W.T), output projection with bias.

Sharding over 8 NeuronCores: batch (4) x head-group (2 groups of 8 heads).
Each core computes q/k/v for its 8 heads of its batch, causal attention in
the S^T layout (keys on partitions, queries on free dim; softmax denominators
produced by an appended ones-column in V), then a PARTIAL output projection
over its own 8 heads (contraction 512) for all 2048 queries. The host sums
the two partial projections of each batch (the tensor-parallel all-reduce,
done host-side because on-device collectives on this fabric run ~30 GB/s);
ATTN_COMBINE=rs switches to an on-device chunked pairwise ReduceScatter.

Single fused pipeline: attention steps t=(head, qh, j) emit S(t) on the PE
and exp(t) on ACT, with AV(t-1) lagged one step. The QKV projection is NOT a
separate phase: K/Q/V tile computations are interleaved as "filler" PE work
inside the attention step stream (the attention loop is ACT-bound, so the PE
slack absorbs them), sharing the PSUM s-ring. kT is stored zero-padded per
head on the full 128 partitions so every matmul in the kernel runs in the
same (128,128) PE tile mode — mode switches would expose every LDWEIGHTS.

Everything is bf16 into fp32 PSUM; partial projections are exchanged fp32.
"""
import os
import sys
import types

import numpy as np

import concourse.bass as bass
import concourse.mybir as mybir
import concourse.tile as tile
from concourse import bacc, bass_utils

DT = getattr(mybir.dt, os.environ.get("ATTN_DT_MAIN", "bfloat16"))
F32 = mybir.dt.float32
AF = mybir.ActivationFunctionType
OP = mybir.AluOpType

B, T, D = 4, 2048, 1024
H, HD = 16, 64
HG = 8          # heads per core
QH = T // 2     # query half
N_CORES = 8
SCALE = 1.0 / 8.0

ADT_NAME = os.environ.get("ATTN_DTYPE", "bfloat16")
ADT = getattr(mybir.dt, ADT_NAME)
COMBINE = os.environ.get("ATTN_COMBINE", "hostsum")

RG_PAIRS = [[0, 1], [2, 3], [4, 5], [6, 7]]


# ---------------------------------------------------------------------------
# environment glue
# ---------------------------------------------------------------------------

def _install_ntff_hook():
    if 'antenv.axon_hooks' in sys.modules:
        return
    try:
        from trn_agent_boot.trn_boot import _ntff_profile_via_ctypes
        hook = _ntff_profile_via_ctypes('/opt/axon/libaxon_pjrt.so')
    except Exception:
        hook = None
    mod = types.ModuleType('antenv.axon_hooks')
    mod.get_axon_ntff_profile_hook = lambda: hook
    mod.set_axon_ntff_profile_hook = lambda h: None
    sys.modules['antenv.axon_hooks'] = mod


def _run_spmd(nc, in_maps, trace=False):
    from concourse.bass_interp import get_hw_module
    bass_utils.upload_artifacts = lambda tmpdir: tmpdir
    if trace:
        _install_ntff_hook()
    old_m = nc.m
    nc.m = get_hw_module(nc.m)
    try:
        return bass_utils.run_bass_kernel_spmd(
            nc, in_maps, core_ids=list(range(N_CORES)),
            trace=trace, trace_cores=[0] if trace else None,
        )
    finally:
        nc.m = old_m


# ---------------------------------------------------------------------------
# kernel program
# ---------------------------------------------------------------------------

def build_nc(combine):
    nc = bacc.Bacc("TRN2", target_bir_lowering=False, debug=False,
                   enable_asserts=False, num_devices=N_CORES)
    xT = nc.dram_tensor("xT", [D, T], DT, kind="ExternalInput").ap()
    wqT = nc.dram_tensor("wqT", [D, 512], DT, kind="ExternalInput").ap()
    wkT = nc.dram_tensor("wkT", [D, 512], DT, kind="ExternalInput").ap()
    wvT = nc.dram_tensor("wvT", [D, 512], DT, kind="ExternalInput").ap()
    wpT = nc.dram_tensor("wpT", [512, D], DT, kind="ExternalInput").ap()
    mask = nc.dram_tensor("mask", [128, 128], ADT, kind="ExternalInput").ap()
    zro = nc.dram_tensor("zro", [1, T], ADT, kind="ExternalInput").ap()
    bias = nc.dram_tensor("bias", [1, D], F32, kind="ExternalInput").ap()
    snum = nc.dram_tensor("snum", [32, QH], F32).ap()
    srec = nc.dram_tensor("srec", [32, QH], ADT).ap()
    if combine == "rs":
        rsin = [nc.dram_tensor(f"rsin{i}", [2, 128, D], F32).ap()
                for i in range(8)]
        yint = nc.dram_tensor("yint", [8, 128, D], F32).ap()
        yo = nc.dram_tensor("yo", [8, 128, D], F32, kind="ExternalOutput").ap()
    else:
        yo = nc.dram_tensor("yo", [T, D], F32, kind="ExternalOutput").ap()

    from contextlib import ExitStack
    with tile.TileContext(nc) as tc, ExitStack() as ctx:
        per = ctx.enter_context(tc.tile_pool(name="per", bufs=1))

        qT_sb = per.tile([128, 4, T], ADT, tag="qT")
        kT_sb = per.tile([128, 8, T], ADT, tag="kT")
        v_sb = per.tile([128, 16, HG, 65], ADT, tag="v")
        mask_sb = per.tile([128, 128], ADT, tag="mask")
        o_all = per.tile([128, 4, T], ADT, tag="oacc")
        wp_sb = per.tile([128, 4, D], DT, tag="wp")
        bias_bc = per.tile([128, D], F32, tag="bbc")

        nc.sync.dma_start(mask_sb[:], mask[:])

        # zero the dead half of a head's kT slot via a broadcast DMA
        # (gpsimd memset signals completion unreliably -> races with S reads)
        def emit_zro(h):
            dead = slice(64, 128) if h % 2 == 0 else slice(0, 64)
            nc.sync.dma_start(kT_sb[dead, h, :],
                              zro[0][None, :].broadcast_to([64, T]))

        xp = ctx.enter_context(tc.tile_pool(name="xph", bufs=16))
        wpo = ctx.enter_context(tc.tile_pool(name="wph", bufs=24))
        xT_r = xT.rearrange("(ko ki) t -> ki ko t", ki=128)

        def load_w(wT, nm):
            parts = []
            wT_r = wT.rearrange("(ko ki) n -> ki ko n", ki=128)
            for kk in range(8):
                t = wpo.tile([128, 512], DT, tag="w", name=f"{nm}{kk}")
                nc.sync.dma_start(t[:], wT_r[:, kk])
                parts.append(t)
            return parts

        def load_x(half):
            out = []
            for kk in range(8):
                t = xp.tile([128, QH], DT, tag="xh", name=f"x{half}_{kk}")
                nc.sync.dma_start(t[:], xT_r[:, kk, half * QH:(half + 1) * QH])
                out.append(t)
            return out

        # input DMAs in consumption order: K(0,0) needs wk + x half0 first
        wk_sb = load_w(wkT, "wk")
        xh = [None, None]
        xh[0] = load_x(0)
        wq_sb = load_w(wqT, "wq")
        emit_zro(0)
        emit_zro(1)
        xh[1] = load_x(1)
        wv_sb = load_w(wvT, "wv")
        nc.sync.dma_start(wp_sb[:],
                          wpT.rearrange("(ko ki) n -> ki ko n", ki=128))
        nc.sync.dma_start(bias_bc[:], bias[0][None, :].broadcast_to([128, D]))

        yop = ctx.enter_context(tc.tile_pool(name="yop", bufs=3))
        with ExitStack() as attn:
            sps = attn.enter_context(tc.tile_pool(name="sps", bufs=2, space="PSUM"))
            ops = attn.enter_context(tc.tile_pool(name="ops", bufs=2, space="PSUM"))
            es = attn.enter_context(tc.tile_pool(name="es", bufs=10))
            ev = attn.enter_context(tc.tile_pool(name="ev", bufs=2))
            nrm = attn.enter_context(tc.tile_pool(name="nrm", bufs=2))

            # ---- filler units: K/Q/V tile computations fed into PE slack --
            def emit_K(m, half):
                # kT for heads (2m, 2m+1), zero-padded layout
                pt = sps.tile([128, QH], F32, tag="s", name=f"ptk{m}_{half}")
                for nch in range(2):
                    sl = slice(nch * 512, (nch + 1) * 512)
                    for kk in range(8):
                        nc.tensor.matmul(
                            pt[:, sl],
                            lhsT=wk_sb[kk][:, m * 128:(m + 1) * 128],
                            rhs=xh[half][kk][:, sl],
                            start=(kk == 0), stop=(kk == 7))
                tsl = slice(half * QH, (half + 1) * QH)
                nc.vector.tensor_copy(kT_sb[0:64, 2 * m, tsl], pt[0:64, :])
                nc.vector.tensor_copy(kT_sb[64:128, 2 * m + 1, tsl],
                                      pt[64:128, :])

            def emit_Q(m, half):
                pt = sps.tile([128, QH], F32, tag="s", name=f"ptq{m}_{half}")
                for nch in range(2):
                    sl = slice(nch * 512, (nch + 1) * 512)
                    for kk in range(8):
                        nc.tensor.matmul(
                            pt[:, sl],
                            lhsT=wq_sb[kk][:, m * 128:(m + 1) * 128],
                            rhs=xh[half][kk][:, sl],
                            start=(kk == 0), stop=(kk == 7))
                nc.vector.tensor_copy(
                    qT_sb[:, m, half * QH:(half + 1) * QH], pt[:])

            def emit_V(m):
                # V' for key block m, all 8 heads (+ ones column for the
                # softmax denominators)
                pt = sps.tile([128, QH], F32, tag="s", name=f"ptv{m}")
                for kk in range(8):
                    nc.tensor.matmul(
                        pt[:, 0:512],
                        lhsT=xh[m // 8][kk][:, (m % 8) * 128:(m % 8 + 1) * 128],
                        rhs=wv_sb[kk][:],
                        start=(kk == 0), stop=(kk == 7))
                nc.vector.tensor_copy(
                    v_sb[:, m, :, 0:64],
                    pt[:, 0:512].rearrange("p (h d) -> p h d", h=HG))
                nc.vector.memset(v_sb[:, m, :, 64], 1.0)

            # ---- attention steps -----------------------------------------
            # Head h's qh0 triangle is interleaved 2:1 into head (h-1)'s qh1
            # block: smooths the exp sizes ACT sees, removes head-transition
            # bubbles, and finishes every qh0 norm before the last qh1 block
            # (so half the projection can fill the drain).
            steps = [(0, 0, j) for j in range(8)]
            for h in range(HG - 1):
                for i in range(8):
                    steps.append((h, 1, 2 * i))
                    steps.append((h, 1, 2 * i + 1))
                    steps.append((h + 1, 0, i))
            steps += [(HG - 1, 1, j) for j in range(16)]
            n_steps = len(steps)

            e_tiles = [None] * n_steps
            o_tiles = {}

            def emit_S_exp(t):
                h, qh, j = steps[t]
                qstart = max(QH * qh, 128 * j)
                n = QH * (qh + 1) - qstart
                sub = h // 2
                s_ps = sps.tile([128, QH], F32, tag="s", name=f"s{t}")
                for c in range(0, n, 512):
                    cn = min(512, n - c)
                    nc.tensor.matmul(
                        s_ps[:, c:c + cn],
                        lhsT=kT_sb[:, h, j * 128:(j + 1) * 128],
                        rhs=qT_sb[:, sub, qstart + c:qstart + c + cn],
                        start=True, stop=True)
                e_sb = es.tile([128, QH], ADT, tag="e", name=f"e{t}")
                nc.scalar.activation(e_sb[:, 0:n], s_ps[:, 0:n], AF.Exp,
                                     scale=SCALE)
                if j >= 8 * qh:
                    nc.vector.tensor_tensor(
                        e_sb[:, 0:128], e_sb[:, 0:128], mask_sb[:], OP.mult)
                e_tiles[t] = (e_sb, n, qstart - QH * qh)

            def emit_AV(t):
                h, qh, j = steps[t]
                e_sb, n, coff = e_tiles[t]
                if j == 0:
                    o_tiles[(h, qh)] = ops.tile([65, QH], F32, tag="o",
                                                name=f"o{h}_{qh}")
                o_ps = o_tiles[(h, qh)]
                jmax = 8 * qh + 8
                c0 = coff
                while c0 < QH:
                    hi = min(QH, (c0 // 512 + 1) * 512)
                    nc.tensor.matmul(
                        o_ps[:, c0:hi],
                        lhsT=v_sb[:, j, h, :],
                        rhs=e_sb[:, c0 - coff:hi - coff],
                        start=(j == 0), stop=(j == jmax - 1),
                        skip_group_check=True)
                    c0 = hi
                e_tiles[t] = None

            def emit_evict(h, qh):
                o_ps = o_tiles.pop((h, qh))
                pbase = 64 * (h % 2)
                sub = h // 2
                i = 2 * h + qh
                dtile = ev.tile([1, QH], F32, tag="dn", name=f"dn{h}_{qh}")
                nc.vector.tensor_copy(dtile[:], o_ps[64:65, :])
                nc.sync.dma_start(snum[i:i + 1, :], dtile[:])
                if pbase == 0:
                    nc.vector.tensor_copy(
                        o_all[0:64, sub, QH * qh:QH * (qh + 1)], o_ps[0:64, :])
                else:
                    tmp = ev.tile([64, QH], ADT, tag="ev", name=f"ev{h}_{qh}")
                    nc.vector.tensor_copy(tmp[:], o_ps[0:64, :])
                    nc.sync.dma_start(
                        o_all[64:128, sub, QH * qh:QH * (qh + 1)], tmp[:])
                st64 = nrm.tile([64, QH // 64], F32, tag="sp")
                nc.sync.dma_start(st64[:], snum[i].rearrange("(p f) -> p f", p=64))
                nc.vector.reciprocal(st64[:], st64[:])
                st64b = nrm.tile([64, QH // 64], ADT, tag="spb")
                nc.vector.tensor_copy(st64b[:], st64[:])
                nc.sync.dma_start(srec[i].rearrange("(p f) -> p f", p=64), st64b[:])
                bc = nrm.tile([128, QH], ADT, tag="bc")
                nc.sync.dma_start(bc[pbase:pbase + 64, :],
                                  srec[i][None, :].broadcast_to([64, QH]))
                sl_ap = o_all[pbase:pbase + 64, sub, QH * qh:QH * (qh + 1)]
                nc.vector.tensor_tensor(sl_ap, sl_ap, bc[pbase:pbase + 64, :],
                                        OP.mult)

            # ---- filler schedule: emission position -> list of closures --
            fillers = {}

            def add_filler(pos, fn, *args):
                fillers.setdefault(pos, []).append((fn, args))

            # ---- projection tiles (shared psum ring) ---------------------
            def emit_proj(m, pool, ptag="s"):
                yp = pool.tile([128, QH], F32, tag=ptag, name=f"yp{m}")
                for nch in range(2):
                    sl = slice(nch * 512, (nch + 1) * 512)
                    for kk in range(4):
                        nc.tensor.matmul(
                            yp[:, sl],
                            lhsT=o_all[:, kk, m * 128:(m + 1) * 128],
                            rhs=wp_sb[:, kk, sl],
                            start=(kk == 0), stop=(kk == 3))
                y_sb = yop.tile([128, D], F32, tag="y")
                nc.vector.tensor_tensor(y_sb[:], yp[:], bias_bc[:], OP.add)
                if combine == "rs":
                    nc.sync.dma_start(rsin[m % 8][m // 8], y_sb[:])
                else:
                    nc.sync.dma_start(yo[m * 128:(m + 1) * 128, :], y_sb[:])

            # AV is lagged LAG steps behind S/exp so it never waits on ACT
            # and its ldweights hides under the S streams; this also relaxes
            # every V-filler deadline so QKV work spreads across the span.
            LAG = 8
            for j, pos in zip(range(8), (1, 2, 4, 5, 7, 8, 10, 11)):
                add_filler(pos, emit_V, j)              # AV(h0,qh0,j) at j+LAG
            for idx, j in enumerate(range(8, 16)):
                add_filler(13 + 2 * idx, emit_V, j)     # AV(h0,qh1,j) at j+8+LAG
            add_filler(3, emit_Q, 0, 1)                 # h0 qh1 from step 8
            add_filler(6, emit_K, 0, 1)                 # h0 qh1 j>=8 by step 20
            for p in (1, 2, 3):
                add_filler(48 * p - 20, emit_K, p, 0)   # by step 48p-14
                add_filler(48 * p - 16, emit_Q, p, 0)   # by step 48p-14
                add_filler(48 * p + 2, emit_Q, p, 1)    # by step 48p+8
                add_filler(48 * p + 12, emit_K, p, 1)   # by step 48p+20
            for h in range(2, HG):
                add_filler(24 * h - 20, emit_zro, h)    # by step 24h-14
            for idx, m in enumerate(range(8)):
                add_filler(184 + idx, emit_proj, m, sps)  # qh0 norms @~183

            # ---- fused emission ------------------------------------------
            emit_K(0, 0)
            emit_Q(0, 0)
            for t in range(n_steps):
                emit_S_exp(t)
                for fn, args in fillers.get(t, []):
                    fn(*args)
                if t >= LAG:
                    emit_AV(t - LAG)
                    h0, qh0, j0 = steps[t - LAG]
                    if j0 == 8 * qh0 + 7:
                        emit_evict(h0, qh0)
            for t in range(n_steps - LAG, n_steps):
                emit_AV(t)
                h0, qh0, j0 = steps[t]
                if j0 == 8 * qh0 + 7:
                    emit_evict(h0, qh0)

        pps = ctx.enter_context(tc.tile_pool(name="pps", bufs=3, space="PSUM"))
        for m in range(8, 16):
            emit_proj(m, pps, ptag="yp")
        if combine == "rs":
            for i in range(8):
                nc.gpsimd.collective_compute(
                    "ReduceScatter", OP.add,
                    replica_groups=RG_PAIRS,
                    ins=[rsin[i][:]], outs=[yint[i]],
                )
                nc.sync.dma_start(yo[i], yint[i])

    nc.compile()
    return nc


# ---------------------------------------------------------------------------
# host-side sharding + entry point
# ---------------------------------------------------------------------------

_NC_CACHE = {}


def _get_nc(combine):
    if combine not in _NC_CACHE:
        _NC_CACHE[combine] = build_nc(combine)
    return _NC_CACHE[combine]


def _make_in_maps(x, Wq, Wk, Wv, Wp, bp):
    x = np.asarray(x, dtype=np.float32)
    Wq = np.asarray(Wq, dtype=np.float32)
    Wk = np.asarray(Wk, dtype=np.float32)
    Wv = np.asarray(Wv, dtype=np.float32)
    Wp = np.asarray(Wp, dtype=np.float32)
    bp = np.asarray(bp, dtype=np.float32)

    adt_np = mybir.dt.np(ADT)
    dt_np = mybir.dt.np(DT)
    mask = np.zeros((128, 128), dtype=np.float32)
    k_idx = np.arange(128)[:, None]
    q_idx = np.arange(128)[None, :]
    mask[q_idx >= k_idx] = 1.0
    mask = mask.astype(adt_np)

    xTs = [np.ascontiguousarray(x[b].T) for b in range(B)]
    in_maps = []
    for c in range(N_CORES):
        b, g = c // 2, c % 2
        rows = slice(512 * g, 512 * (g + 1))
        m = {
            "xT": xTs[b].astype(dt_np),
            "wqT": np.ascontiguousarray(Wq[rows, :].T).astype(dt_np),
            "wkT": np.ascontiguousarray(Wk[rows, :].T).astype(dt_np),
            "wvT": np.ascontiguousarray(Wv[rows, :].T).astype(dt_np),
            "wpT": np.ascontiguousarray(Wp[:, rows].T).astype(dt_np),
            "mask": mask,
            "zro": np.zeros((1, T), dtype=adt_np),
            "bias": (bp if g == 0 else np.zeros_like(bp)).reshape(1, D),
        }
        in_maps.append(m)
    return in_maps


def kernel(x, Wq, Wk, Wv, Wp, bp, _trace=False):
    combine = COMBINE
    nc = _get_nc(combine)
    in_maps = _make_in_maps(x, Wq, Wk, Wv, Wp, bp)
    res = _run_spmd(nc, in_maps, trace=_trace)
    out = np.empty((B, T, D), dtype=np.float32)
    for b in range(B):
        ya = res.results[2 * b]["yo"]
        yb = res.results[2 * b + 1]["yo"]
        if combine == "rs":
            out[b, 0:QH] = ya.reshape(QH, D)
            out[b, QH:T] = yb.reshape(QH, D)
        else:
            out[b] = ya + yb
    if _trace:
        kernel.last_results = res
    return out
